# revision 21
# baseline (speedup 1.0000x reference)
"""Trainium2 Bass kernel for BlockPrototypeMemory (sparse block attention).

Reference computation (fp32):
  mem = 4-layer MLP(mem_params)            [1, P, NB, DB]
  mem = block_ln(mem); q = block_ln(queries)
  scores = einsum('bnhd,zmhd->bhnm', q*DB^-.5, mem)
  out = softmax(scores) @ mem              [B, N, D]

Sharding: tensor-parallel over the NB=16 blocks; each of the 8 cores owns
2 blocks (a 256-wide slice of D) for all batches/tokens.  No collectives:
the host concatenates the 8 per-core [B*N, 256] outputs along D.  Queries,
MLP weights and mem_params are converted to bf16 on the host (halves DMA
and removes in-kernel weight casts) and the output is returned as bf16
and widened on the host.

Per-core algorithm (blocks h=0,1):
  Phase 0: MLP on mem_params.T slices (weights pre-transposed host-side),
    block-LN of the P=512 prototype rows, build kv_aug [keys, DB | 1.0]
    (ones column accumulates the softmax denominator inside the PV matmul)
    and kT [DB, keys] scaled by DB^-0.5 (folds the q-side scale).
    Block h=0 is emitted first, then batch-0 LN stats and the first q
    transposes, then block h=1 - so the attention pipeline starts primed
    ~25us in instead of serializing behind all of phase 0.
  Phase 1: software-pipelined units of (group=512 tokens, block h, key
    half v): scores for 2 key chunks -> one [128,2,512] PSUM tile (2
    banks, 3-slot rotation = 6 banks), exp of that tile on ACT
    ([128,1024] per call keeps ACT ~95% busy - it is the binding engine),
    then per h a 16-matmul PV pass into two 1-bank [128,2,129] PSUM
    tiles (ones column = denominator), normalize on DVE, store bf16.
    PE program order runs pv(k) before sc(k+2) so the in-order PE queue
    never head-of-line blocks on a psum slot that frees mid-period.
    Mean subtraction of q is dropped: LN'd keys are zero-mean, so mu_q
    contributes nothing to scores.  exp needs no max subtraction:
    |s| <= sqrt(DB) so exp <= e^11.4, safe in fp32/bf16.
  LN stats for q: one DVE bn_stats pass per 4 segments (even/odd split
  stats combined analytically), istd via Newton rsqrt on DVE only.
  Batch b+1's q rows are loaded during batch b-1 and its stats run in
  the first half of batch b, so neither a fresh q DMA nor the istd
  finalization ever gates a qprep or blocks the DVE queue (norms) at a
  batch boundary.  q transposes (Pool scale + DMA transpose, a ~5.3us
  chain) are issued 4 groups ahead of their scores unit.
"""

import numpy as np

import concourse.bass as bass
import concourse.bacc as bacc
import concourse.mybir as mybir
import concourse.tile as tile
from concourse.bass import ts
from concourse.bass_utils import run_bass_kernel_spmd

F32 = mybir.dt.float32
BF16 = mybir.dt.bfloat16
I32 = mybir.dt.int32
ALU = mybir.AluOpType
ACT = mybir.ActivationFunctionType
AX = mybir.AxisListType

# ---- problem dims ----
B, N, D = 8, 4096, 2048
NB, DB = 16, 128
P = 512            # prototypes (attention keys)
HID = 4 * DB       # 512 MLP hidden
NCORES = 8
HPC = NB // NCORES  # 2 blocks per core
DS = HPC * DB       # 256 per-core D slice
EPS = 1e-5
KC = P // 128       # 4 key chunks
JC = HID // 128     # 4 hidden chunks


def _rsqrt(nc, pool, x, out, F, tag):
    """out = 1/sqrt(x) elementwise on DVE only (no ACT table swaps).

    Quake seed + 3 Newton iterations: ~1e-7 rel err for x in (1e-6, 1e3).
    x, out: [128, F] f32 APs (may alias views of dense tiles).
    """
    ti = pool.tile([128, F], I32, tag=tag + "_i")
    t2 = pool.tile([128, F], F32, tag=tag + "_t")
    y = pool.tile([128, F], F32, tag=tag + "_y")
    magic = pool.tile([128, 1], I32, tag=tag + "_m")
    nc.vector.memset(magic[:], 0x5F3759DF)
    nc.vector.tensor_scalar(ti[:], x.bitcast(I32), 1, None, op0=ALU.arith_shift_right)
    nc.vector.tensor_tensor(ti[:], magic[:].broadcast_to([128, F]), ti[:], ALU.subtract)
    yv = ti[:].bitcast(F32)
    for it in range(3):
        dst = out if it == 2 else y[:]
        nc.vector.tensor_tensor(t2[:], yv, yv, ALU.mult)
        nc.vector.tensor_tensor(t2[:], t2[:], x, ALU.mult)
        nc.vector.tensor_scalar(t2[:], t2[:], -0.5, 1.5, op0=ALU.mult, op1=ALU.add)
        nc.vector.tensor_tensor(dst, yv, t2[:], ALU.mult)
        yv = y[:]


def build_nc(nbb=B, nt=N // 512):
    """Build the per-core Bass module. nbb batches x nt 512-token groups."""
    nc = bacc.Bacc("TRN2", target_bir_lowering=False, debug=False)
    rows = nbb * nt * 512
    nsub = 4 * nt          # 128-row subtiles per batch
    nseg = nsub * HPC      # LN segments per batch (tokens%128 x (s,h))

    q_d = nc.dram_tensor("q", [rows, DS], BF16, kind="ExternalInput").ap()
    mpt_d = nc.dram_tensor("mpt", [HPC, DB, P], BF16, kind="ExternalInput").ap()
    w1t_d = nc.dram_tensor("w1t", [DB, HID], BF16, kind="ExternalInput").ap()
    w2t_d = nc.dram_tensor("w2t", [HID, HID], BF16, kind="ExternalInput").ap()
    w3t_d = nc.dram_tensor("w3t", [HID, HID], BF16, kind="ExternalInput").ap()
    w4t_d = nc.dram_tensor("w4t", [HID, DB], BF16, kind="ExternalInput").ap()
    b1r_d = nc.dram_tensor("b1r", [DB, JC], F32, kind="ExternalInput").ap()
    b2r_d = nc.dram_tensor("b2r", [DB, JC], F32, kind="ExternalInput").ap()
    b3r_d = nc.dram_tensor("b3r", [DB, JC], F32, kind="ExternalInput").ap()
    b4rep_d = nc.dram_tensor("b4rep", [DB, DB], F32, kind="ExternalInput").ap()
    out_d = nc.dram_tensor("out", [rows, DS], BF16, kind="ExternalOutput").ap()

    ngroups = nbb * nt
    nunits = 2 * ngroups
    SC_LA = 2     # sc/exp emitted this many units ahead of pv
    QP_LA = 6     # qprep emitted this many groups ahead of the sc stream
    nchunks = nseg // 4
    NLOAD = max(1, nsub // 8)

    q_v = q_d.rearrange("(b s p) d -> b p s d", b=nbb, p=128)
    out_v = out_d.rearrange("(b t s p) (h e) -> b t p s h e",
                            b=nbb, s=4, p=128, h=HPC)

    with tile.TileContext(nc) as tc:
        with (
            tc.tile_pool(name="const", bufs=1) as const,
            tc.tile_pool(name="qres", bufs=3) as qres_p,
            tc.tile_pool(name="stat", bufs=3) as stat_p,
            tc.tile_pool(name="sfin", bufs=2) as sfin_p,
            tc.tile_pool(name="qbf", bufs=8) as qbf_p,
            tc.tile_pool(name="qT", bufs=8) as qT_p,
            tc.tile_pool(name="E", bufs=10) as e_p,
            tc.tile_pool(name="ob", bufs=4) as ob_p,
            tc.tile_pool(name="rec", bufs=8) as rec_p,
        ):
            # ---- bf16 weights straight from DRAM ----
            w1t_b = const.tile([DB, HID], BF16)
            w2t_b = const.tile([128, JC, HID], BF16)
            w3t_b = const.tile([128, JC, HID], BF16)
            w4t_b = const.tile([128, JC, DB], BF16)
            b1r = const.tile([DB, JC], F32)
            b2r = const.tile([DB, JC], F32)
            b3r = const.tile([DB, JC], F32)
            b4rep = const.tile([DB, DB], F32)
            nc.sync.dma_start(w1t_b[:], w1t_d)
            nc.sync.dma_start(w2t_b[:], w2t_d.rearrange("(c p) o -> p c o", p=128))
            nc.sync.dma_start(w3t_b[:], w3t_d.rearrange("(c p) o -> p c o", p=128))
            nc.sync.dma_start(w4t_b[:], w4t_d.rearrange("(c p) o -> p c o", p=128))
            nc.sync.dma_start(b1r[:], b1r_d)
            nc.sync.dma_start(b2r[:], b2r_d)
            nc.sync.dma_start(b3r[:], b3r_d)
            nc.sync.dma_start(b4rep[:], b4rep_d)

            # keys: kv_aug rows (unscaled, + ones col) and kT (DB^-0.5 folded)
            kv_aug = const.tile([128, HPC, KC, 130], BF16)
            kT = const.tile([DB, HPC, P], BF16)
            nc.vector.memset(kv_aug[:, :, :, 128:130], 0.0)
            nc.vector.memset(kv_aug[:, :, :, 128:129], 1.0)

            # ---- phase-1 state + emitters ----
            state = {}
            qTs = {}
            ees = {}
            obs = {}

            def emit_load(b, i):
                # one staggered slice of batch b's q rows (i in 0..NLOAD-1)
                if i == 0:
                    qr = qres_p.tile([128, nsub, DS], BF16, tag="qr",
                                     name=f"qr{b}")
                    state[b] = {
                        "qr": qr,
                        "bst": stat_p.tile([128, nseg, 6], F32, tag="bst",
                                           name=f"bst{b}"),
                        "istd": stat_p.tile([128, nsub, HPC], F32,
                                            tag="qistd", name=f"istd{b}"),
                    }
                qr = state[b]["qr"]
                w = nsub // NLOAD
                nc.sync.dma_start(qr[:, i * w:(i + 1) * w, :],
                                  q_v[b, :, i * w:(i + 1) * w, :])

            def emit_stats_chunk(b, j):
                # bn_stats: per-128-segment even/odd stats in one DVE pass
                st = state[b]
                qrv = st["qr"][:].rearrange("p s (h e) -> p (s h) e", e=DB)
                nc.vector.bn_stats(st["bst"][:, 4 * j:4 * j + 4, :],
                                   qrv[:, 4 * j:4 * j + 4, :])

            def emit_stats_final(b, half=None):
                # half=None: all segments; half=0/1: low/high half (keeps
                # the DVE burst short enough to not delay queued norms)
                st = state[b]
                lo, hi = (0, nseg) if half is None else (
                    half * nseg // 2, (half + 1) * nseg // 2)
                n = hi - lo
                bst = st["bst"][:, lo:hi, :]
                # var = (cv_e + cv_o)/DB + ((m_e - m_o)/2)^2  (+EPS)
                d = sfin_p.tile([128, n], F32, tag="sd")
                cv = sfin_p.tile([128, n], F32, tag="scv")
                d2 = sfin_p.tile([128, n], F32, tag="sd2")
                var = sfin_p.tile([128, n], F32, tag="svar")
                nc.vector.tensor_tensor(d[:], bst[:, :, 1], bst[:, :, 4],
                                        ALU.subtract)
                nc.vector.tensor_tensor(cv[:], bst[:, :, 2], bst[:, :, 5],
                                        ALU.add)
                nc.vector.tensor_tensor(d2[:], d[:], d[:], ALU.mult)
                nc.vector.tensor_scalar(cv[:], cv[:], 1.0 / DB, EPS,
                                        op0=ALU.mult, op1=ALU.add)
                nc.vector.scalar_tensor_tensor(
                    out=var[:], in0=d2[:], scalar=0.25, in1=cv[:],
                    op0=ALU.mult, op1=ALU.add)
                _rsqrt(nc, sfin_p, var[:],
                       st["istd"][:].rearrange("p s h -> p (s h)")[:, lo:hi],
                       n, "rsq")

            def emit_qprep(g):
                # qb = q * istd (Pool), then DMA-transpose to qT [DB, tok]
                b, t = divmod(g, nt)
                qr = state[b]["qr"]
                istd = state[b]["istd"]
                qb = qbf_p.tile([128, HPC, 4, DB], BF16, tag="qb",
                                name=f"qb{g}")
                nc.gpsimd.tensor_tensor(
                    qb[:],
                    qr[:, 4 * t:4 * t + 4, :].rearrange(
                        "p s (h e) -> p h s e", e=DB),
                    istd[:, 4 * t:4 * t + 4, :].rearrange(
                        "p s h -> p h s")[:, :, :, None].broadcast_to(
                        [128, HPC, 4, DB]),
                    ALU.mult)
                qT = qT_p.tile([128, HPC, 512], BF16, tag="qT",
                               name=f"qT{g}")
                nc.sync.dma_start_transpose(
                    qT[:].rearrange("p h (s e) -> p (h s) e", e=128),
                    qb[:].rearrange("p h s e -> p (h s e)"))
                qTs[g] = qT

            # ---- phase 0: mem MLP + LN per block ----
            with (
                tc.tile_pool(name="mlp_ps", bufs=2, space="PSUM") as mlp_ps,
                tc.tile_pool(name="mlp_ps4", bufs=2, space="PSUM") as mlp_ps4,
                tc.tile_pool(name="mlp_sb", bufs=2) as mlp_sb,
            ):
                def emit_mlp(h):
                    x_b = mlp_sb.tile([DB, P], BF16, tag="xb")
                    nc.sync.dma_start(x_b[:], mpt_d[h])
                    h1 = mlp_sb.tile([128, JC, P], BF16, tag="h1")
                    for j in range(JC):
                        ps = mlp_ps.tile([128, P], F32, tag="ps")
                        nc.tensor.matmul(ps[:], w1t_b[:, ts(j, 128)], x_b[:],
                                         start=True, stop=True)
                        nc.scalar.activation(h1[:, j, :], ps[:], ACT.Relu,
                                             bias=b1r[:, j:j + 1])
                    h2 = mlp_sb.tile([128, JC, P], BF16, tag="h2")
                    for j in range(JC):
                        ps = mlp_ps.tile([128, P], F32, tag="ps")
                        for i in range(JC):
                            nc.tensor.matmul(ps[:], w2t_b[:, i, ts(j, 128)],
                                             h1[:, i, :],
                                             start=(i == 0), stop=(i == JC - 1))
                        nc.scalar.activation(h2[:, j, :], ps[:], ACT.Relu,
                                             bias=b2r[:, j:j + 1])
                    h3 = mlp_sb.tile([128, JC, P], BF16, tag="h3")
                    for j in range(JC):
                        ps = mlp_ps.tile([128, P], F32, tag="ps")
                        for i in range(JC):
                            nc.tensor.matmul(ps[:], w3t_b[:, i, ts(j, 128)],
                                             h2[:, i, :],
                                             start=(i == 0), stop=(i == JC - 1))
                        nc.scalar.activation(h3[:, j, :], ps[:], ACT.Relu,
                                             bias=b3r[:, j:j + 1])
                    # L4 in row layout [keys, DB] so LN stats are free-dim
                    m_f = mlp_sb.tile([128, KC, DB], F32, tag="mf")
                    for r in range(KC):
                        ps4 = mlp_ps4.tile([128, DB], F32, tag="ps4")
                        for i in range(JC):
                            nc.tensor.matmul(ps4[:], h3[:, i, ts(r, 128)],
                                             w4t_b[:, i, :],
                                             start=(i == 0), stop=(i == JC - 1))
                        nc.vector.tensor_tensor(m_f[:, r, :], ps4[:], b4rep[:],
                                                ALU.add)
                    # block-LN over DB (free dim) for the 4 row chunks
                    msum = mlp_sb.tile([128, KC], F32, tag="msum")
                    msq = mlp_sb.tile([128, KC], F32, tag="msq")
                    scr = mlp_sb.tile([128, KC, DB], F32, tag="scr")
                    nc.vector.reduce_sum(msum[:], m_f[:], axis=AX.X)
                    nc.scalar.activation(scr[:], m_f[:], ACT.Square)
                    nc.vector.reduce_sum(msq[:], scr[:], axis=AX.X)
                    mu = mlp_sb.tile([128, KC], F32, tag="mu")
                    var = mlp_sb.tile([128, KC], F32, tag="var")
                    nc.vector.tensor_scalar(mu[:], msum[:], 1.0 / DB, None,
                                            op0=ALU.mult)
                    nc.vector.tensor_scalar(var[:], msq[:], 1.0 / DB, None,
                                            op0=ALU.mult)
                    nc.vector.tensor_tensor(scr[:, 0, :KC], mu[:], mu[:],
                                            ALU.mult)
                    nc.vector.tensor_tensor(var[:], var[:], scr[:, 0, :KC],
                                            ALU.subtract)
                    nc.vector.tensor_scalar(var[:], var[:], EPS, None,
                                            op0=ALU.add)
                    istd = mlp_sb.tile([128, KC], F32, tag="istd")
                    istd_s = mlp_sb.tile([128, KC], F32, tag="istds")
                    _rsqrt(nc, mlp_sb, var[:], istd[:], KC, "rsm")
                    nc.vector.tensor_scalar(istd_s[:], istd[:], DB ** -0.5,
                                            None, op0=ALU.mult)
                    kvs = mlp_sb.tile([128, KC, DB], BF16, tag="kvs")
                    for r in range(KC):
                        nc.vector.scalar_tensor_tensor(
                            out=kv_aug[:, h, r, :128], in0=m_f[:, r, :],
                            scalar=mu[:, r:r + 1],
                            in1=istd[:, r:r + 1].broadcast_to([128, DB]),
                            op0=ALU.subtract, op1=ALU.mult)
                        nc.vector.scalar_tensor_tensor(
                            out=kvs[:, r, :], in0=m_f[:, r, :],
                            scalar=mu[:, r:r + 1],
                            in1=istd_s[:, r:r + 1].broadcast_to([128, DB]),
                            op0=ALU.subtract, op1=ALU.mult)
                    nc.sync.dma_start_transpose(
                        kT[:, h, :].rearrange("p (s e) -> p s e", e=128),
                        kvs[:].rearrange("p s e -> p (s e)"))

                # interleave phase 0 with batch-0 stats + first q preps so
                # the attention pipeline starts primed right after phase 0
                for i in range(NLOAD):
                    emit_load(0, i)
                if nbb > 1:
                    for i in range(NLOAD):
                        emit_load(1, i)
                emit_mlp(0)
                half = max(1, nchunks // 2)
                for j in range(half):
                    emit_stats_chunk(0, j)
                emit_stats_final(0, 0)
                for g in range(min(4, ngroups)):
                    emit_qprep(g)
                for j in range(half, nchunks):
                    emit_stats_chunk(0, j)
                emit_stats_final(0, 1)
                for g in range(4, min(QP_LA + 1, ngroups)):
                    emit_qprep(g)
                emit_mlp(1)

            # ---- phase 1: attention units ----
            with (
                tc.tile_pool(name="scps", bufs=2, space="PSUM") as sc_ps,
                tc.tile_pool(name="pvps", bufs=4, space="PSUM") as pv_ps,
            ):
                def emit_sc_exp(k):
                    # scores + exp for unit k = (group, h), per key half v
                    g, h = divmod(k, 2)
                    qT = qTs[g]
                    ees[k] = []
                    for v in range(2):
                        pss = sc_ps.tile([128, 2, 512], F32, tag="sc",
                                         name=f"sc{k}_{v}")
                        for i in range(2):
                            nc.tensor.matmul(pss[:, i, :],
                                             kT[:, h, ts(2 * v + i, 128)],
                                             qT[:, h, :], start=True, stop=True)
                        ee = e_p.tile([128, 2, 512], BF16, tag="ee",
                                      name=f"ee{k}_{v}")
                        nc.scalar.activation(ee[:], pss[:], ACT.Exp)
                        ees[k].append(ee)

                pending_out = []

                def emit_pv(k):
                    # PV + normalize (+ store after h=1), one pipeline stage
                    # behind sc/exp.  16 MMs per unit; two 1-bank psum tiles
                    # whose s-groups run back-to-back (no interleaved psum
                    # accumulation groups within a bank).
                    g, h = divmod(k, 2)
                    b, t = divmod(g, nt)
                    if h == 0:
                        obs[g] = ob_p.tile([128, 4, HPC, DB], BF16, tag="ob",
                                           name=f"ob{g}")
                    ob = obs[g]
                    for sp in range(2):
                        pvt = pv_ps.tile([128, 2, 129], F32, tag="pv",
                                         name=f"pv{k}_{sp}")
                        for sl in range(2):
                            s = 2 * sp + sl
                            for c in range(KC):
                                nc.tensor.matmul(
                                    pvt[:, sl, :129],
                                    ees[k][c // 2][:, c % 2, ts(s, 128)],
                                    kv_aug[:, h, c, :129],
                                    start=(c == 0), stop=(c == KC - 1))
                        rec = rec_p.tile([128, 2, 1], F32, tag="rec")
                        nc.vector.reciprocal(rec[:], pvt[:, :, 128:129])
                        nc.vector.tensor_tensor(
                            ob[:, 2 * sp:2 * sp + 2, h, :], pvt[:, :, :128],
                            rec[:].broadcast_to([128, 2, DB]), ALU.mult)
                    if h == 1:
                        # defer the store by one group: when SP reaches it,
                        # the norm it waits on is long done, so it can never
                        # head-of-line block the qT transpose dispatches
                        pending_out.append((b, t, ob))
                        del obs[g]
                        del ees[k - 1]
                        del ees[k]

                def flush_out(keep=0):
                    # keep: leave this many newest entries pending (deferral
                    # depth in groups) so the store's norm-wait is already
                    # satisfied when the in-order SP queue reaches it
                    while len(pending_out) > keep:
                        bb, tt, ob = pending_out.pop(0)
                        nc.sync.dma_start(out_v[bb, tt], ob[:])

                for k in range(min(SC_LA, nunits)):
                    emit_sc_exp(k)

                # ---- steady loop: pv(k) first, then sc(k+SC_LA) ----
                # pv-first keeps the in-order PE queue from head-of-line
                # blocking: sc(k+2,v1)'s psum slot frees mid-period, so if
                # it were queued ahead of pv(k) the PE would stall on it
                # even though pv(k) is already runnable.
                for k in range(nunits):
                    g, h = divmod(k, 2)
                    b, t = divmod(g, nt)
                    kl = k - 2 * b * nt       # unit index within batch b
                    kk = k + SC_LA
                    if kk < nunits and kk % 2 == 0:
                        gp = kk // 2 + QP_LA
                        if gp < ngroups:
                            emit_qprep(gp)
                    if k % 2 == 0:
                        flush_out(keep=1)     # stores deferred two groups
                    emit_pv(k)
                    if b + 2 < nbb and kl >= 2 * nt - 6:
                        i = kl - (2 * nt - 6)
                        if i < NLOAD:
                            emit_load(b + 2, i)
                    if b + 1 < nbb:
                        # four bn_stats chunks per unit over the first four
                        # units; finals right after each half's chunks so
                        # istd(b+1) is ready for the deep qprep lookahead
                        j0 = 4 * kl
                        if j0 < nchunks:
                            for j in range(j0, min(j0 + 4, nchunks)):
                                emit_stats_chunk(b + 1, j)
                        if j0 + 4 == nchunks // 2 or (nchunks <= 4 and kl == 0):
                            emit_stats_final(b + 1, 0)
                        elif j0 + 4 == nchunks:
                            emit_stats_final(b + 1, 1)
                    if kk < nunits:
                        emit_sc_exp(kk)
                flush_out()
    nc.compile()
    return nc


_CACHE = {}


def _get_nc(nbb, nt):
    key = (nbb, nt)
    if key not in _CACHE:
        _CACHE[key] = build_nc(nbb, nt)
    return _CACHE[key]


def make_in_maps(queries, mem_params, w1, b1, w2, b2, w3, b3, w4, b4):
    import ml_dtypes
    f = np.float32
    bf = ml_dtypes.bfloat16
    shared = {
        "w1t": np.ascontiguousarray(np.asarray(w1, f).T.astype(bf)),
        "w2t": np.ascontiguousarray(np.asarray(w2, f).T.astype(bf)),
        "w3t": np.ascontiguousarray(np.asarray(w3, f).T.astype(bf)),
        "w4t": np.ascontiguousarray(np.asarray(w4, f).T.astype(bf)),
        "b1r": np.ascontiguousarray(np.asarray(b1, f).reshape(JC, DB).T),
        "b2r": np.ascontiguousarray(np.asarray(b2, f).reshape(JC, DB).T),
        "b3r": np.ascontiguousarray(np.asarray(b3, f).reshape(JC, DB).T),
        "b4rep": np.ascontiguousarray(np.tile(np.asarray(b4, f), (DB, 1))),
    }
    qbf = np.asarray(queries, f).astype(bf)
    nbb, ntok, dd = queries.shape
    in_maps = []
    for c in range(NCORES):
        mp = np.asarray(mem_params, f)[0, :, c * HPC:(c + 1) * HPC, :]  # [P,HPC,DB]
        m = dict(shared)
        m["q"] = np.ascontiguousarray(
            qbf[:, :, c * DS:(c + 1) * DS]).reshape(-1, DS)
        m["mpt"] = np.ascontiguousarray(
            mp.transpose(1, 2, 0).astype(bf))  # [HPC, DB, P]
        in_maps.append(m)
    return in_maps


def kernel(queries, mem_params, w1, b1, w2, b2, w3, b3, w4, b4):
    queries = np.asarray(queries, np.float32)
    nbb, ntok, dd = queries.shape
    nt = ntok // 512
    nc = _get_nc(nbb, nt)
    in_maps = make_in_maps(queries, mem_params, w1, b1, w2, b2, w3, b3, w4, b4)
    res = run_bass_kernel_spmd(nc, in_maps, list(range(NCORES))).results
    out = np.concatenate(
        [np.asarray(res[c]["out"], np.float32).reshape(nbb, ntok, DS)
         for c in range(NCORES)], axis=-1)
    return np.ascontiguousarray(out, dtype=np.float32)


if __name__ == "__main__":
    nc = build_nc(1, 1)
    print("built ok")
